# revision 10
# baseline (speedup 1.0000x reference)
"""Location-sensitive attention on 8 Trainium2 NeuronCores.

Strategy (data-parallel over batch, 4 rows/core):
  - Host stages the big memory tensor per-core in two fp16 layouts:
      memT16 [4, E, T]  (E-major, feeds the W_memory projection: contraction
                         over E must sit on SBUF partitions)
      mem16  [4, T, E]  (natural, feeds the context matvec: contraction over T)
    Total HBM traffic per core = 2 x 16 MiB = 32 MiB, the same byte count as
    reading the fp32 input once; all matmul contractions land on partitions
    with zero on-chip transposes of the big tensor.
  - Energies are accumulated in PSUM [A=128, T-tile] (pm + conv-location
    projection), tanh+query-bias applied by ScalarE, reduced against W_value
    by TensorE into [T,1] columns, softmax done in a [128, 32] layout with a
    cross-partition renorm via two tiny PE transposes.
  - conv(prev_attn, prev_attn_cum) is folded with W_loc on the host into a
    single [62, A] projection applied to a 62-row stack of shifted windows,
    built by overlapping-read DMAs from host-padded rows.
"""

import numpy as np

import concourse.bass as bass
import concourse.tile as tile
from concourse import bacc, mybir
from concourse.bass_utils import run_bass_kernel_spmd

B, T, E, Q = 32, 4096, 512, 1024
A, F, KW = 128, 32, 31
NCORES = 8
BPC = B // NCORES          # batch rows per core
PAD = 15                   # SAME conv padding for kernel width 31
TPADLEN = T + 2 * PAD + 2  # 4128, padded to a multiple of 32
NG = T // 128              # 32 column-groups in the [128, NG] energy layout
NSUPER = T // 512          # 8 T-supertiles of 512
f32 = mybir.dt.float32
f16 = mybir.dt.float16

TRACE = False              # test.py flips this to collect an NTFF profile
_CACHE = {}


def _build():
    nc = bacc.Bacc(
        "TRN2", target_bir_lowering=False, debug=False, num_devices=NCORES
    )

    memT16 = nc.dram_tensor("memT16", [BPC, E, T], f16, kind="ExternalInput")
    mem16 = nc.dram_tensor("mem16", [BPC, T, E], f16, kind="ExternalInput")
    wm16 = nc.dram_tensor("wm16", [128, 4, 128], f16, kind="ExternalInput")
    wcomb = nc.dram_tensor("wcomb", [2, 31, A], f32, kind="ExternalInput")
    pqT = nc.dram_tensor("pqT", [A, BPC], f32, kind="ExternalInput")
    wv = nc.dram_tensor("wv", [A, 1], f32, kind="ExternalInput")
    prevpad = nc.dram_tensor("prevpad", [BPC, TPADLEN], f32, kind="ExternalInput")
    cumpad = nc.dram_tensor("cumpad", [BPC, TPADLEN], f32, kind="ExternalInput")
    maskb = nc.dram_tensor("maskb", [BPC, 128, NG], f32, kind="ExternalInput")
    cumT = nc.dram_tensor("cumT", [BPC, 128, NG], f32, kind="ExternalInput")
    ident = nc.dram_tensor("ident", [128, 128], f32, kind="ExternalInput")

    ctx_out = nc.dram_tensor("ctx_out", [BPC, E], f32, kind="ExternalOutput")
    attnT_out = nc.dram_tensor("attnT_out", [BPC, 128, NG], f32, kind="ExternalOutput")
    cumT_out = nc.dram_tensor("cumT_out", [BPC, 128, NG], f32, kind="ExternalOutput")

    with tile.TileContext(nc) as tc:
        with (
            tc.tile_pool(name="consts", bufs=1) as consts,
            tc.tile_pool(name="memt", bufs=8) as memt_pool,
            tc.tile_pool(name="mem2", bufs=3) as mem2_pool,
            tc.tile_pool(name="catp", bufs=2) as cat_pool,
            tc.tile_pool(name="zp", bufs=3) as z_pool,
            tc.tile_pool(name="small", bufs=2) as small,
            tc.tile_pool(name="ps_pm", bufs=2, space="PSUM") as ps_pm,
            tc.tile_pool(name="ps_e", bufs=2, space="PSUM") as ps_e,
            tc.tile_pool(name="ps_ctx", bufs=2, space="PSUM") as ps_ctx,
            tc.tile_pool(name="ps_st", bufs=2, space="PSUM") as ps_st,
        ):
            wm_sb = consts.tile([128, 4, 128], f16)
            nc.sync.dma_start(out=wm_sb, in_=wm16[:, :, :])
            wc_sb = consts.tile([31, 2, A], f32)
            nc.sync.dma_start(out=wc_sb, in_=wcomb.rearrange("c k a -> k c a"))
            pq_sb = consts.tile([A, BPC], f32)
            nc.sync.dma_start(out=pq_sb, in_=pqT[:, :])
            wv_sb = consts.tile([A, 1], f32)
            nc.sync.dma_start(out=wv_sb, in_=wv[:, :])
            id_sb = consts.tile([128, 128], f32)
            nc.sync.dma_start(out=id_sb, in_=ident[:, :])
            mk_sb = consts.tile([128, BPC, NG], f32)
            nc.sync.dma_start(out=mk_sb, in_=maskb.rearrange("b p g -> p b g"))
            cm_sb = consts.tile([128, BPC, NG], f32)
            nc.sync.dma_start(out=cm_sb, in_=cumT.rearrange("b p g -> p b g"))

            for b in range(BPC):
                # ---------------- pass 1: energies ----------------
                energ = small.tile([128, NG], f32, tag="energ")
                for half in range(2):
                    t0 = 2048 * half
                    mts = []
                    for j in range(4):
                        mt = memt_pool.tile([128, 2048], f16, tag="mt")
                        nc.sync.dma_start(
                            out=mt, in_=memT16[b, 128 * j:128 * (j + 1), t0:t0 + 2048]
                        )
                        mts.append(mt)
                    cat = cat_pool.tile([31, 2, 2048], f32, tag="cat")
                    for ci, src_t in enumerate((prevpad, cumpad)):
                        base = src_t[b, t0:t0 + 2048]
                        ov = bass.AP(
                            tensor=base.tensor,
                            offset=base.offset,
                            ap=[[1, 31], [1, 2048]],
                        )
                        nc.sync.dma_start(out=cat[:, ci, :], in_=ov)
                    for g4 in range(4):
                        sup = 4 * half + g4
                        c0 = 512 * g4
                        pm = ps_pm.tile([128, 512], f32, tag="pm")
                        for j in range(4):
                            nc.tensor.matmul(
                                pm, wm_sb[:, j, :], mts[j][:, c0:c0 + 512],
                                start=(j == 0), stop=False,
                            )
                        for ci in range(2):
                            nc.tensor.matmul(
                                pm, wc_sb[:, ci, :], cat[:, ci, c0:c0 + 512],
                                start=False, stop=(ci == 1),
                            )
                        z = z_pool.tile([128, 512], f32, tag="z")
                        nc.scalar.activation(
                            out=z, in_=pm, func=mybir.ActivationFunctionType.Tanh,
                            bias=pq_sb[:, b:b + 1],
                        )
                        pe4 = ps_e.tile([128, 4], f32, tag="pe")
                        for s in range(4):
                            nc.tensor.matmul(
                                pe4[:, s:s + 1], z[:, 128 * s:128 * (s + 1)], wv_sb,
                                start=(s == 0), stop=(s == 3),
                            )
                        nc.vector.tensor_copy(
                            out=energ[:, 4 * sup:4 * sup + 4], in_=pe4
                        )

                # ---------------- softmax over T (layout [128, NG]) ----------
                nc.vector.tensor_add(energ, energ, mk_sb[:, b, :])
                mrow = small.tile([128, 1], f32, tag="mrow")
                nc.vector.reduce_max(out=mrow, in_=energ, axis=mybir.AxisListType.X)
                negm = small.tile([128, 1], f32, tag="negm")
                nc.vector.tensor_scalar_mul(negm, mrow, -1.0)
                ssum = small.tile([128, 1], f32, tag="ssum")
                eexp = small.tile([128, NG], f32, tag="eexp")
                nc.scalar.activation(
                    out=eexp, in_=energ, func=mybir.ActivationFunctionType.Exp,
                    bias=negm, accum_out=ssum,
                )
                pn = ps_st.tile([1, 128], f32, tag="st")
                nc.tensor.transpose(pn, negm, id_sb)
                psrow = ps_st.tile([1, 128], f32, tag="st")
                nc.tensor.transpose(psrow, ssum, id_sb)
                nrow = small.tile([1, 128], f32, tag="nrow")
                nc.vector.tensor_copy(out=nrow, in_=pn)
                srow = small.tile([1, 128], f32, tag="srow")
                nc.vector.tensor_copy(out=srow, in_=psrow)
                minneg = small.tile([1, 1], f32, tag="minneg")
                nc.vector.tensor_reduce(
                    out=minneg, in_=nrow, axis=mybir.AxisListType.X,
                    op=mybir.AluOpType.min,
                )
                wrow = small.tile([1, 128], f32, tag="wrow")
                nc.scalar.activation(
                    out=wrow, in_=nrow,
                    func=mybir.ActivationFunctionType.Exp,
                    bias=minneg, scale=-1.0,
                )
                swrow = small.tile([1, 128], f32, tag="swrow")
                nc.vector.tensor_mul(swrow, wrow, srow)
                stot = small.tile([1, 1], f32, tag="stot")
                nc.vector.reduce_sum(
                    out=stot, in_=swrow, axis=mybir.AxisListType.X
                )
                rinv = small.tile([1, 1], f32, tag="rinv")
                nc.vector.reciprocal(out=rinv, in_=stot)
                nc.vector.tensor_scalar_mul(wrow, wrow, rinv)
                pf = ps_st.tile([128, 1], f32, tag="st")
                nc.tensor.transpose(pf, wrow, id_sb[0:1, 0:1])
                fcol = small.tile([128, 1], f32, tag="fcol")
                nc.vector.tensor_copy(out=fcol, in_=pf)

                attnf = small.tile([128, NG], f32, tag="attnf")
                nc.vector.tensor_scalar_mul(attnf, eexp, fcol)
                nc.sync.dma_start(out=attnT_out[b, :, :], in_=attnf)
                attn16 = small.tile([128, NG], f16, tag="attn16")
                nc.vector.tensor_copy(out=attn16, in_=attnf)
                ncum = small.tile([128, NG], f32, tag="ncum")
                nc.vector.tensor_add(ncum, attnf, cm_sb[:, b, :])
                nc.sync.dma_start(out=cumT_out[b, :, :], in_=ncum)

                # ---------------- pass 2: context ----------------
                pc = ps_ctx.tile([1, 512], f32, tag="pc")
                nat = mem16[b, :, :].rearrange("(g t) e -> t g e", t=128)
                for h2 in range(4):
                    m2 = mem2_pool.tile([128, 8, 512], f16, tag="m2")
                    nc.sync.dma_start(out=m2, in_=nat[:, 8 * h2:8 * h2 + 8, :])
                    for i in range(8):
                        g = 8 * h2 + i
                        nc.tensor.matmul(
                            pc, attn16[:, g:g + 1], m2[:, i, :],
                            start=(g == 0), stop=(g == 31),
                        )
                ctx_sb = small.tile([1, 512], f32, tag="ctx_sb")
                nc.vector.tensor_copy(out=ctx_sb, in_=pc)
                nc.sync.dma_start(out=ctx_out[b, :], in_=ctx_sb)

    nc.compile()
    return nc


def _get_nc():
    if "nc" not in _CACHE:
        _CACHE["nc"] = _build()
    return _CACHE["nc"]


def kernel(query, memory, mask, prev_attn, prev_attn_cum,
           W_query, W_memory, W_value, conv_kernel, W_loc):
    query = np.asarray(query, np.float32)
    memory = np.asarray(memory, np.float32)
    mask = np.asarray(mask)
    prev_attn = np.asarray(prev_attn, np.float32)
    prev_attn_cum = np.asarray(prev_attn_cum, np.float32)
    W_query = np.asarray(W_query, np.float32)
    W_memory = np.asarray(W_memory, np.float32)
    W_value = np.asarray(W_value, np.float32)
    conv_kernel = np.asarray(conv_kernel, np.float32)
    W_loc = np.asarray(W_loc, np.float32)

    # Tiny host-side staging: query projection, conv+W_loc fold, padding and
    # the [128, NG] relayouts that match the on-chip energy layout.
    pq = query @ W_query                                   # [B, A]
    wcomb = np.einsum("kcf,fa->cka", conv_kernel, W_loc)  # [2, 31, A]
    wm16 = np.ascontiguousarray(
        W_memory.astype(np.float16).reshape(4, 128, 128).transpose(1, 0, 2)
    )
    mem16 = memory.astype(np.float16)                      # [B, T, E]
    memT16 = memory.transpose(0, 2, 1).astype(np.float16)  # [B, E, T]
    prevpad = np.zeros((B, TPADLEN), np.float32)
    prevpad[:, PAD:PAD + T] = prev_attn
    cumpad = np.zeros((B, TPADLEN), np.float32)
    cumpad[:, PAD:PAD + T] = prev_attn_cum
    fmin = np.float32(np.finfo(np.float32).min)
    maskb = np.ascontiguousarray(
        np.where(mask, np.float32(0.0), fmin).reshape(B, NG, 128).transpose(0, 2, 1)
    )
    cumT = np.ascontiguousarray(
        prev_attn_cum.reshape(B, NG, 128).transpose(0, 2, 1)
    )
    ident = np.eye(128, dtype=np.float32)
    wv = np.ascontiguousarray(W_value.reshape(A, 1))

    nc = _get_nc()
    in_maps = []
    for c in range(NCORES):
        sl = slice(c * BPC, (c + 1) * BPC)
        in_maps.append({
            "memT16": np.ascontiguousarray(memT16[sl]),
            "mem16": np.ascontiguousarray(mem16[sl]),
            "wm16": wm16,
            "wcomb": np.ascontiguousarray(wcomb),
            "pqT": np.ascontiguousarray(pq[sl].T),
            "wv": wv,
            "prevpad": np.ascontiguousarray(prevpad[sl]),
            "cumpad": np.ascontiguousarray(cumpad[sl]),
            "maskb": maskb[sl],
            "cumT": cumT[sl],
            "ident": ident,
        })

    res = run_bass_kernel_spmd(nc, in_maps, list(range(NCORES)), trace=TRACE)
    _CACHE["last_result"] = res

    ctx = np.concatenate([r["ctx_out"] for r in res.results], axis=0)
    attnT = np.concatenate([r["attnT_out"] for r in res.results], axis=0)
    ncumT = np.concatenate([r["cumT_out"] for r in res.results], axis=0)
    attn = np.ascontiguousarray(attnT.transpose(0, 2, 1)).reshape(B, T)
    new_cum = np.ascontiguousarray(ncumT.transpose(0, 2, 1)).reshape(B, T)
    return ctx, attn, new_cum


# revision 15
# speedup vs baseline: 11681.2182x; 11681.2182x over previous
"""Location-sensitive attention on 8 Trainium2 NeuronCores.

Strategy (data-parallel over batch, 4 rows/core):
  - Host stages the big memory tensor per-core in two fp16 layouts:
      memT16 [4, E, T]  (E-major, feeds the W_memory projection: contraction
                         over E must sit on SBUF partitions)
      mem16  [4, T, E]  (natural, feeds the context matvec: contraction over T)
    Total HBM traffic per core = 2 x 16 MiB = 32 MiB, the same byte count as
    reading the fp32 input once; all matmul contractions land on partitions
    with zero on-chip transposes of the big tensor.
  - Energies are accumulated in PSUM [A=128, T-tile] (pm + conv-location
    projection), tanh+query-bias applied by ScalarE, reduced against W_value
    by TensorE into [T,1] columns, softmax done in a [128, 32] layout with a
    cross-partition renorm via two tiny PE transposes.
  - conv(prev_attn, prev_attn_cum) is folded with W_loc on the host into a
    single [62, A] projection applied to a 62-row stack of shifted windows,
    built by overlapping-read DMAs from host-padded rows.
"""

import numpy as np

import concourse.bass as bass
import concourse.tile as tile
from concourse import bacc, mybir
from concourse.bass_utils import run_bass_kernel_spmd

B, T, E, Q = 32, 4096, 512, 1024
A, F, KW = 128, 32, 31
NCORES = 8
BPC = B // NCORES          # batch rows per core
PAD = 15                   # SAME conv padding for kernel width 31
TPADLEN = T + 2 * PAD + 2  # 4128, padded to a multiple of 32
NG = T // 128              # 32 column-groups in the [128, NG] energy layout
NSUPER = T // 512          # 8 T-supertiles of 512
f32 = mybir.dt.float32
f16 = mybir.dt.float16

TRACE = False              # test.py flips this to collect an NTFF profile
_CACHE = {}


def _build(reps=1):
    nc = bacc.Bacc(
        "TRN2", target_bir_lowering=False, debug=False, num_devices=NCORES
    )

    memT16 = nc.dram_tensor("memT16", [BPC, E, T], f16, kind="ExternalInput")
    mem16 = nc.dram_tensor("mem16", [BPC, T, E], f16, kind="ExternalInput")
    wm16 = nc.dram_tensor("wm16", [128, 4, 128], f16, kind="ExternalInput")
    wcomb = nc.dram_tensor("wcomb", [2, 31, A], f32, kind="ExternalInput")
    pqT = nc.dram_tensor("pqT", [A, BPC], f32, kind="ExternalInput")
    wv = nc.dram_tensor("wv", [A, 1], f32, kind="ExternalInput")
    prevpad = nc.dram_tensor("prevpad", [BPC, TPADLEN], f32, kind="ExternalInput")
    cumpad = nc.dram_tensor("cumpad", [BPC, TPADLEN], f32, kind="ExternalInput")
    maskb = nc.dram_tensor("maskb", [BPC, 128, NG], f32, kind="ExternalInput")
    cumT = nc.dram_tensor("cumT", [BPC, 128, NG], f32, kind="ExternalInput")
    ident = nc.dram_tensor("ident", [128, 128], f32, kind="ExternalInput")

    ctx_out = nc.dram_tensor("ctx_out", [BPC, E], f32, kind="ExternalOutput")
    attnT_out = nc.dram_tensor("attnT_out", [BPC, 128, NG], f32, kind="ExternalOutput")
    cumT_out = nc.dram_tensor("cumT_out", [BPC, 128, NG], f32, kind="ExternalOutput")

    with tile.TileContext(nc) as tc:
        with (
            tc.tile_pool(name="consts", bufs=1) as consts,
            tc.tile_pool(name="memt", bufs=8) as memt_pool,
            tc.tile_pool(name="mem2", bufs=3) as mem2_pool,
            tc.tile_pool(name="catp", bufs=2) as cat_pool,
            tc.tile_pool(name="zp", bufs=3) as z_pool,
            tc.tile_pool(name="small", bufs=2) as small,
            tc.tile_pool(name="ps_pm", bufs=2, space="PSUM") as ps_pm,
            tc.tile_pool(name="ps_e", bufs=2, space="PSUM") as ps_e,
            tc.tile_pool(name="ps_ctx", bufs=2, space="PSUM") as ps_ctx,
            tc.tile_pool(name="ps_st", bufs=2, space="PSUM") as ps_st,
        ):
            wm_sb = consts.tile([128, 4, 128], f16)
            nc.sync.dma_start(out=wm_sb, in_=wm16[:, :, :])
            wc_sb = consts.tile([31, 2, A], f32)
            nc.sync.dma_start(out=wc_sb, in_=wcomb.rearrange("c k a -> k c a"))
            pq_sb = consts.tile([A, BPC], f32)
            nc.sync.dma_start(out=pq_sb, in_=pqT[:, :])
            wv_sb = consts.tile([A, 1], f32)
            nc.sync.dma_start(out=wv_sb, in_=wv[:, :])
            id_sb = consts.tile([128, 128], f32)
            nc.sync.dma_start(out=id_sb, in_=ident[:, :])
            mk_sb = consts.tile([128, BPC, NG], f32)
            nc.sync.dma_start(out=mk_sb, in_=maskb.rearrange("b p g -> p b g"))
            cm_sb = consts.tile([128, BPC, NG], f32)
            nc.sync.dma_start(out=cm_sb, in_=cumT.rearrange("b p g -> p b g"))

            for b in [b for _ in range(reps) for b in range(BPC)]:
                # ---------------- pass 1: energies ----------------
                energ = small.tile([128, NG], f32, tag="energ")
                for half in range(2):
                    t0 = 2048 * half
                    mts = []
                    for j in range(4):
                        mt = memt_pool.tile([128, 2048], f16, tag="mt")
                        nc.sync.dma_start(
                            out=mt, in_=memT16[b, 128 * j:128 * (j + 1), t0:t0 + 2048]
                        )
                        mts.append(mt)
                    cat = cat_pool.tile([31, 2, 2048], f32, tag="cat")
                    for ci, src_t in enumerate((prevpad, cumpad)):
                        base = src_t[b, t0:t0 + 2048]
                        ov = bass.AP(
                            tensor=base.tensor,
                            offset=base.offset,
                            ap=[[1, 31], [1, 2048]],
                        )
                        nc.sync.dma_start(out=cat[:, ci, :], in_=ov)
                    for g4 in range(4):
                        sup = 4 * half + g4
                        c0 = 512 * g4
                        pm = ps_pm.tile([128, 512], f32, tag="pm")
                        for j in range(4):
                            nc.tensor.matmul(
                                pm, wm_sb[:, j, :], mts[j][:, c0:c0 + 512],
                                start=(j == 0), stop=False,
                            )
                        for ci in range(2):
                            nc.tensor.matmul(
                                pm, wc_sb[:, ci, :], cat[:, ci, c0:c0 + 512],
                                start=False, stop=(ci == 1),
                            )
                        z = z_pool.tile([128, 512], f32, tag="z")
                        nc.scalar.activation(
                            out=z, in_=pm, func=mybir.ActivationFunctionType.Tanh,
                            bias=pq_sb[:, b:b + 1],
                        )
                        pe4 = ps_e.tile([128, 4], f32, tag="pe")
                        for s in range(4):
                            nc.tensor.matmul(
                                pe4[:, s:s + 1], z[:, 128 * s:128 * (s + 1)], wv_sb,
                                start=(s == 0), stop=(s == 3),
                            )
                        nc.vector.tensor_copy(
                            out=energ[:, 4 * sup:4 * sup + 4], in_=pe4
                        )

                # ---------------- softmax over T (layout [128, NG]) ----------
                nc.vector.tensor_add(energ, energ, mk_sb[:, b, :])
                mrow = small.tile([128, 1], f32, tag="mrow")
                nc.vector.reduce_max(out=mrow, in_=energ, axis=mybir.AxisListType.X)
                negm = small.tile([128, 1], f32, tag="negm")
                nc.vector.tensor_scalar_mul(negm, mrow, -1.0)
                ssum = small.tile([128, 1], f32, tag="ssum")
                eexp = small.tile([128, NG], f32, tag="eexp")
                nc.scalar.activation(
                    out=eexp, in_=energ, func=mybir.ActivationFunctionType.Exp,
                    bias=negm, accum_out=ssum,
                )
                pn = ps_st.tile([1, 128], f32, tag="st")
                nc.tensor.transpose(pn, negm, id_sb)
                psrow = ps_st.tile([1, 128], f32, tag="st")
                nc.tensor.transpose(psrow, ssum, id_sb)
                nrow = small.tile([1, 128], f32, tag="nrow")
                nc.vector.tensor_copy(out=nrow, in_=pn)
                srow = small.tile([1, 128], f32, tag="srow")
                nc.vector.tensor_copy(out=srow, in_=psrow)
                minneg = small.tile([1, 1], f32, tag="minneg")
                nc.vector.tensor_reduce(
                    out=minneg, in_=nrow, axis=mybir.AxisListType.X,
                    op=mybir.AluOpType.min,
                )
                wrow = small.tile([1, 128], f32, tag="wrow")
                nc.scalar.activation(
                    out=wrow, in_=nrow,
                    func=mybir.ActivationFunctionType.Exp,
                    bias=minneg, scale=-1.0,
                )
                swrow = small.tile([1, 128], f32, tag="swrow")
                nc.vector.tensor_mul(swrow, wrow, srow)
                stot = small.tile([1, 1], f32, tag="stot")
                nc.vector.reduce_sum(
                    out=stot, in_=swrow, axis=mybir.AxisListType.X
                )
                rinv = small.tile([1, 1], f32, tag="rinv")
                nc.vector.reciprocal(out=rinv, in_=stot)
                nc.vector.tensor_scalar_mul(wrow, wrow, rinv)
                pf = ps_st.tile([128, 1], f32, tag="st")
                nc.tensor.transpose(pf, wrow, id_sb[0:1, 0:1])
                fcol = small.tile([128, 1], f32, tag="fcol")
                nc.vector.tensor_copy(out=fcol, in_=pf)

                attnf = small.tile([128, NG], f32, tag="attnf")
                nc.vector.tensor_scalar_mul(attnf, eexp, fcol)
                nc.sync.dma_start(out=attnT_out[b, :, :], in_=attnf)
                attn16 = small.tile([128, NG], f16, tag="attn16")
                nc.vector.tensor_copy(out=attn16, in_=attnf)
                ncum = small.tile([128, NG], f32, tag="ncum")
                nc.vector.tensor_add(ncum, attnf, cm_sb[:, b, :])
                nc.sync.dma_start(out=cumT_out[b, :, :], in_=ncum)

                # ---------------- pass 2: context ----------------
                pc = ps_ctx.tile([1, 512], f32, tag="pc")
                nat = mem16[b, :, :].rearrange("(g t) e -> t g e", t=128)
                for h2 in range(4):
                    m2 = mem2_pool.tile([128, 8, 512], f16, tag="m2")
                    nc.sync.dma_start(out=m2, in_=nat[:, 8 * h2:8 * h2 + 8, :])
                    for i in range(8):
                        g = 8 * h2 + i
                        nc.tensor.matmul(
                            pc, attn16[:, g:g + 1], m2[:, i, :],
                            start=(g == 0), stop=(g == 31),
                        )
                ctx_sb = small.tile([1, 512], f32, tag="ctx_sb")
                nc.vector.tensor_copy(out=ctx_sb, in_=pc)
                nc.sync.dma_start(out=ctx_out[b, :], in_=ctx_sb)

    nc.compile()
    return nc


def _get_nc(reps=1):
    key = ("nc", reps)
    if key not in _CACHE:
        _CACHE[key] = _build(reps)
    return _CACHE[key]


def _stage_inputs(query, memory, mask, prev_attn, prev_attn_cum,
                  W_query, W_memory, W_value, conv_kernel, W_loc):
    query = np.asarray(query, np.float32)
    memory = np.asarray(memory, np.float32)
    mask = np.asarray(mask)
    prev_attn = np.asarray(prev_attn, np.float32)
    prev_attn_cum = np.asarray(prev_attn_cum, np.float32)
    W_query = np.asarray(W_query, np.float32)
    W_memory = np.asarray(W_memory, np.float32)
    W_value = np.asarray(W_value, np.float32)
    conv_kernel = np.asarray(conv_kernel, np.float32)
    W_loc = np.asarray(W_loc, np.float32)

    # Tiny host-side staging: query projection, conv+W_loc fold, padding and
    # the [128, NG] relayouts that match the on-chip energy layout.
    pq = query @ W_query                                   # [B, A]
    wcomb = np.einsum("kcf,fa->cka", conv_kernel, W_loc)  # [2, 31, A]
    wm16 = np.ascontiguousarray(
        W_memory.astype(np.float16).reshape(4, 128, 128).transpose(1, 0, 2)
    )
    mem16 = memory.astype(np.float16)                      # [B, T, E]
    memT16 = memory.transpose(0, 2, 1).astype(np.float16)  # [B, E, T]
    prevpad = np.zeros((B, TPADLEN), np.float32)
    prevpad[:, PAD:PAD + T] = prev_attn
    cumpad = np.zeros((B, TPADLEN), np.float32)
    cumpad[:, PAD:PAD + T] = prev_attn_cum
    fmin = np.float32(np.finfo(np.float32).min)
    maskb = np.ascontiguousarray(
        np.where(mask, np.float32(0.0), fmin).reshape(B, NG, 128).transpose(0, 2, 1)
    )
    cumT = np.ascontiguousarray(
        prev_attn_cum.reshape(B, NG, 128).transpose(0, 2, 1)
    )
    ident = np.eye(128, dtype=np.float32)
    wv = np.ascontiguousarray(W_value.reshape(A, 1))

    in_maps = []
    for c in range(NCORES):
        sl = slice(c * BPC, (c + 1) * BPC)
        in_maps.append({
            "memT16": np.ascontiguousarray(memT16[sl]),
            "mem16": np.ascontiguousarray(mem16[sl]),
            "wm16": wm16,
            "wcomb": np.ascontiguousarray(wcomb),
            "pqT": np.ascontiguousarray(pq[sl].T),
            "wv": wv,
            "prevpad": np.ascontiguousarray(prevpad[sl]),
            "cumpad": np.ascontiguousarray(cumpad[sl]),
            "maskb": maskb[sl],
            "cumT": cumT[sl],
            "ident": ident,
        })
    return in_maps


def kernel(query, memory, mask, prev_attn, prev_attn_cum,
           W_query, W_memory, W_value, conv_kernel, W_loc):
    in_maps = _stage_inputs(query, memory, mask, prev_attn, prev_attn_cum,
                            W_query, W_memory, W_value, conv_kernel, W_loc)
    nc = _get_nc()
    res = run_bass_kernel_spmd(nc, in_maps, list(range(NCORES)), trace=TRACE)
    _CACHE["last_result"] = res

    ctx = np.concatenate([r["ctx_out"] for r in res.results], axis=0)
    attnT = np.concatenate([r["attnT_out"] for r in res.results], axis=0)
    ncumT = np.concatenate([r["cumT_out"] for r in res.results], axis=0)
    attn = np.ascontiguousarray(attnT.transpose(0, 2, 1)).reshape(B, T)
    new_cum = np.ascontiguousarray(ncumT.transpose(0, 2, 1)).reshape(B, T)
    return ctx, attn, new_cum


# revision 19
# speedup vs baseline: 23016.4510x; 1.9704x over previous
"""Location-sensitive attention on 8 Trainium2 NeuronCores.

Strategy (data-parallel over batch, 4 rows/core):
  - Host stages the big memory tensor per-core in two fp16 layouts:
      memT16 [4, E, T]  (E-major, feeds the W_memory projection: contraction
                         over E must sit on SBUF partitions)
      mem16  [4, T, E]  (natural, feeds the context matvec: contraction over T)
    Total HBM traffic per core = 2 x 16 MiB = 32 MiB, the same byte count as
    reading the fp32 input once; all matmul contractions land on partitions
    with zero on-chip transposes of the big tensor.
  - Energies are accumulated in PSUM [A=128, T-tile] (pm + conv-location
    projection), tanh+query-bias applied by ScalarE, reduced against W_value
    by TensorE into [T,1] columns, softmax done in a [128, 32] layout with a
    cross-partition renorm via two tiny PE transposes.
  - conv(prev_attn, prev_attn_cum) is folded with W_loc on the host into a
    single [62, A] projection applied to a 62-row stack of shifted windows,
    built by overlapping-read DMAs from host-padded rows.
"""

import numpy as np

import concourse.bass as bass
import concourse.tile as tile
from concourse import bacc, mybir
from concourse.bass_utils import run_bass_kernel_spmd

B, T, E, Q = 32, 4096, 512, 1024
A, F, KW = 128, 32, 31
NCORES = 8
BPC = B // NCORES          # batch rows per core
PAD = 15                   # SAME conv padding for kernel width 31
TPADLEN = T + 2 * PAD + 2  # 4128, padded to a multiple of 32
NG = T // 128              # 32 column-groups in the [128, NG] energy layout
NSUPER = T // 512          # 8 T-supertiles of 512
f32 = mybir.dt.float32
f16 = mybir.dt.float16

TRACE = False              # test.py flips this to collect an NTFF profile
_CACHE = {}


def _build(reps=1, ablate=()):
    no_pass2 = "pass2" in ablate
    no_pass1mm = "pass1mm" in ablate
    no_cat = "cat" in ablate
    nc = bacc.Bacc(
        "TRN2", target_bir_lowering=False, debug=False, num_devices=NCORES
    )

    memT16 = nc.dram_tensor("memT16", [BPC, E, T], f16, kind="ExternalInput")
    mem16 = nc.dram_tensor("mem16", [BPC, T, E], f16, kind="ExternalInput")
    wm16 = nc.dram_tensor("wm16", [128, 4, 128], f16, kind="ExternalInput")
    wcomb = nc.dram_tensor("wcomb", [2, 31, A], f16, kind="ExternalInput")
    pqT = nc.dram_tensor("pqT", [A, BPC], f32, kind="ExternalInput")
    wv = nc.dram_tensor("wv", [A, 1], f16, kind="ExternalInput")
    prevpad = nc.dram_tensor("prevpad", [BPC, TPADLEN], f16, kind="ExternalInput")
    cumpad = nc.dram_tensor("cumpad", [BPC, TPADLEN], f16, kind="ExternalInput")
    maskb = nc.dram_tensor("maskb", [BPC, 128, NG], f32, kind="ExternalInput")
    cumT = nc.dram_tensor("cumT", [BPC, 128, NG], f32, kind="ExternalInput")
    ident = nc.dram_tensor("ident", [128, 128], f32, kind="ExternalInput")

    ctx_out = nc.dram_tensor("ctx_out", [BPC, E], f32, kind="ExternalOutput")
    attnT_out = nc.dram_tensor("attnT_out", [BPC, 128, NG], f32, kind="ExternalOutput")
    cumT_out = nc.dram_tensor("cumT_out", [BPC, 128, NG], f32, kind="ExternalOutput")

    with tile.TileContext(nc) as tc:
        with (
            tc.tile_pool(name="consts", bufs=1) as consts,
            tc.tile_pool(name="memt", bufs=10) as memt_pool,
            tc.tile_pool(name="mem2", bufs=6) as mem2_pool,
            tc.tile_pool(name="catp", bufs=2) as cat_pool,
            tc.tile_pool(name="zp", bufs=3) as z_pool,
            tc.tile_pool(name="small", bufs=2) as small,
            tc.tile_pool(name="ps_pm", bufs=2, space="PSUM") as ps_pm,
            tc.tile_pool(name="ps_e", bufs=2, space="PSUM") as ps_e,
            tc.tile_pool(name="ps_ctx", bufs=2, space="PSUM") as ps_ctx,
            tc.tile_pool(name="ps_st", bufs=2, space="PSUM") as ps_st,
        ):
            wm_sb = consts.tile([128, 4, 128], f16)
            nc.sync.dma_start(out=wm_sb, in_=wm16[:, :, :])
            wc_sb = consts.tile([31, 2, A], f16)
            nc.sync.dma_start(out=wc_sb, in_=wcomb.rearrange("c k a -> k c a"))
            pq_sb = consts.tile([A, BPC], f32)
            nc.sync.dma_start(out=pq_sb, in_=pqT[:, :])
            wv_sb = consts.tile([A, 1], f16)
            nc.sync.dma_start(out=wv_sb, in_=wv[:, :])
            id_sb = consts.tile([128, 128], f32)
            nc.sync.dma_start(out=id_sb, in_=ident[:, :])
            mk_sb = consts.tile([128, BPC, NG], f32)
            nc.sync.dma_start(out=mk_sb, in_=maskb.rearrange("b p g -> p b g"))
            cm_sb = consts.tile([128, BPC, NG], f32)
            nc.sync.dma_start(out=cm_sb, in_=cumT.rearrange("b p g -> p b g"))

            for b in [b for _ in range(reps) for b in range(BPC)]:
                # ---------------- pass 1: energies ----------------
                energ = small.tile([128, NG], f32, tag="energ")
                for half in range(2):
                    t0 = 2048 * half
                    mts = []
                    for j in range(4):
                        mt = memt_pool.tile([128, 2048], f16, tag="mt")
                        nc.sync.dma_start(
                            out=mt, in_=memT16[b, 128 * j:128 * (j + 1), t0:t0 + 2048]
                        )
                        mts.append(mt)
                    cat = cat_pool.tile([31, 2, 2048], f16, tag="cat")
                    if not no_cat:
                        for ci, src_t in enumerate((prevpad, cumpad)):
                            base = src_t[b, t0:t0 + 2048]
                            ov = bass.AP(
                                tensor=base.tensor,
                                offset=base.offset,
                                ap=[[1, 31], [1, 2048]],
                            )
                            nc.sync.dma_start(out=cat[:, ci, :], in_=ov)
                    for g4 in range(4):
                        sup = 4 * half + g4
                        c0 = 512 * g4
                        pm = ps_pm.tile([128, 512], f32, tag="pm")
                        if no_pass1mm:
                            nc.tensor.matmul(
                                pm, wm_sb[:, 0, :], mts[0][:, c0:c0 + 512],
                                start=True, stop=True,
                            )
                        else:
                            for j in range(4):
                                nc.tensor.matmul(
                                    pm, wm_sb[:, j, :], mts[j][:, c0:c0 + 512],
                                    start=(j == 0), stop=False,
                                )
                            for ci in range(2):
                                nc.tensor.matmul(
                                    pm, wc_sb[:, ci, :], cat[:, ci, c0:c0 + 512],
                                    start=False, stop=(ci == 1),
                                )
                        z = z_pool.tile([128, 512], f16, tag="z")
                        nc.scalar.activation(
                            out=z, in_=pm, func=mybir.ActivationFunctionType.Tanh,
                            bias=pq_sb[:, b:b + 1],
                        )
                        pe4 = ps_e.tile([128, 4], f32, tag="pe")
                        for s in range(4):
                            nc.tensor.matmul(
                                pe4[:, s:s + 1], z[:, 128 * s:128 * (s + 1)], wv_sb,
                                start=(s == 0), stop=(s == 3),
                            )
                        nc.vector.tensor_copy(
                            out=energ[:, 4 * sup:4 * sup + 4], in_=pe4
                        )

                # ---------------- softmax over T (layout [128, NG]) ----------
                nc.vector.tensor_add(energ, energ, mk_sb[:, b, :])
                mrow = small.tile([128, 1], f32, tag="mrow")
                nc.vector.reduce_max(out=mrow, in_=energ, axis=mybir.AxisListType.X)
                negm = small.tile([128, 1], f32, tag="negm")
                nc.vector.tensor_scalar_mul(negm, mrow, -1.0)
                ssum = small.tile([128, 1], f32, tag="ssum")
                eexp = small.tile([128, NG], f32, tag="eexp")
                nc.scalar.activation(
                    out=eexp, in_=energ, func=mybir.ActivationFunctionType.Exp,
                    bias=negm, accum_out=ssum,
                )
                pn = ps_st.tile([1, 128], f32, tag="st")
                nc.tensor.transpose(pn, negm, id_sb)
                psrow = ps_st.tile([1, 128], f32, tag="st")
                nc.tensor.transpose(psrow, ssum, id_sb)
                nrow = small.tile([1, 128], f32, tag="nrow")
                nc.vector.tensor_copy(out=nrow, in_=pn)
                srow = small.tile([1, 128], f32, tag="srow")
                nc.vector.tensor_copy(out=srow, in_=psrow)
                minneg = small.tile([1, 1], f32, tag="minneg")
                nc.vector.tensor_reduce(
                    out=minneg, in_=nrow, axis=mybir.AxisListType.X,
                    op=mybir.AluOpType.min,
                )
                wrow = small.tile([1, 128], f32, tag="wrow")
                nc.scalar.activation(
                    out=wrow, in_=nrow,
                    func=mybir.ActivationFunctionType.Exp,
                    bias=minneg, scale=-1.0,
                )
                swrow = small.tile([1, 128], f32, tag="swrow")
                nc.vector.tensor_mul(swrow, wrow, srow)
                stot = small.tile([1, 1], f32, tag="stot")
                nc.vector.reduce_sum(
                    out=stot, in_=swrow, axis=mybir.AxisListType.X
                )
                rinv = small.tile([1, 1], f32, tag="rinv")
                nc.vector.reciprocal(out=rinv, in_=stot)
                nc.vector.tensor_scalar_mul(wrow, wrow, rinv)
                pf = ps_st.tile([128, 1], f32, tag="st")
                nc.tensor.transpose(pf, wrow, id_sb[0:1, 0:1])
                fcol = small.tile([128, 1], f32, tag="fcol")
                nc.vector.tensor_copy(out=fcol, in_=pf)

                attnf = small.tile([128, NG], f32, tag="attnf")
                nc.vector.tensor_scalar_mul(attnf, eexp, fcol)
                nc.sync.dma_start(out=attnT_out[b, :, :], in_=attnf)
                attn16 = small.tile([128, NG], f16, tag="attn16")
                nc.vector.tensor_copy(out=attn16, in_=attnf)
                ncum = small.tile([128, NG], f32, tag="ncum")
                nc.vector.tensor_add(ncum, attnf, cm_sb[:, b, :])
                nc.sync.dma_start(out=cumT_out[b, :, :], in_=ncum)

                # ---------------- pass 2: context ----------------
                if no_pass2:
                    continue
                pc = ps_ctx.tile([1, 512], f32, tag="pc")
                nat = mem16[b, :, :].rearrange("(g t) e -> t g e", t=128)
                for h2 in range(4):
                    m2 = mem2_pool.tile([128, 8, 512], f16, tag="m2")
                    nc.sync.dma_start(out=m2, in_=nat[:, 8 * h2:8 * h2 + 8, :])
                    for i in range(8):
                        g = 8 * h2 + i
                        nc.tensor.matmul(
                            pc, attn16[:, g:g + 1], m2[:, i, :],
                            start=(g == 0), stop=(g == 31),
                        )
                ctx_sb = small.tile([1, 512], f32, tag="ctx_sb")
                nc.vector.tensor_copy(out=ctx_sb, in_=pc)
                nc.sync.dma_start(out=ctx_out[b, :], in_=ctx_sb)

    nc.compile()
    return nc


def _get_nc(reps=1):
    key = ("nc", reps)
    if key not in _CACHE:
        _CACHE[key] = _build(reps)
    return _CACHE[key]


def _stage_inputs(query, memory, mask, prev_attn, prev_attn_cum,
                  W_query, W_memory, W_value, conv_kernel, W_loc):
    query = np.asarray(query, np.float32)
    memory = np.asarray(memory, np.float32)
    mask = np.asarray(mask)
    prev_attn = np.asarray(prev_attn, np.float32)
    prev_attn_cum = np.asarray(prev_attn_cum, np.float32)
    W_query = np.asarray(W_query, np.float32)
    W_memory = np.asarray(W_memory, np.float32)
    W_value = np.asarray(W_value, np.float32)
    conv_kernel = np.asarray(conv_kernel, np.float32)
    W_loc = np.asarray(W_loc, np.float32)

    # Tiny host-side staging: query projection, conv+W_loc fold, padding and
    # the [128, NG] relayouts that match the on-chip energy layout.
    pq = query @ W_query                                   # [B, A]
    wcomb = np.einsum("kcf,fa->cka", conv_kernel, W_loc).astype(np.float16)
    wm16 = np.ascontiguousarray(
        W_memory.astype(np.float16).reshape(4, 128, 128).transpose(1, 0, 2)
    )
    mem16 = memory.astype(np.float16)                      # [B, T, E]
    memT16 = memory.transpose(0, 2, 1).astype(np.float16)  # [B, E, T]
    prevpad = np.zeros((B, TPADLEN), np.float16)
    prevpad[:, PAD:PAD + T] = prev_attn
    cumpad = np.zeros((B, TPADLEN), np.float16)
    cumpad[:, PAD:PAD + T] = prev_attn_cum
    fmin = np.float32(np.finfo(np.float32).min)
    maskb = np.ascontiguousarray(
        np.where(mask, np.float32(0.0), fmin).reshape(B, NG, 128).transpose(0, 2, 1)
    )
    cumT = np.ascontiguousarray(
        prev_attn_cum.reshape(B, NG, 128).transpose(0, 2, 1)
    )
    ident = np.eye(128, dtype=np.float32)
    wv = np.ascontiguousarray(W_value.reshape(A, 1).astype(np.float16))

    in_maps = []
    for c in range(NCORES):
        sl = slice(c * BPC, (c + 1) * BPC)
        in_maps.append({
            "memT16": np.ascontiguousarray(memT16[sl]),
            "mem16": np.ascontiguousarray(mem16[sl]),
            "wm16": wm16,
            "wcomb": np.ascontiguousarray(wcomb),
            "pqT": np.ascontiguousarray(pq[sl].T),
            "wv": wv,
            "prevpad": np.ascontiguousarray(prevpad[sl]),
            "cumpad": np.ascontiguousarray(cumpad[sl]),
            "maskb": maskb[sl],
            "cumT": cumT[sl],
            "ident": ident,
        })
    return in_maps


def kernel(query, memory, mask, prev_attn, prev_attn_cum,
           W_query, W_memory, W_value, conv_kernel, W_loc):
    in_maps = _stage_inputs(query, memory, mask, prev_attn, prev_attn_cum,
                            W_query, W_memory, W_value, conv_kernel, W_loc)
    nc = _get_nc()
    res = run_bass_kernel_spmd(nc, in_maps, list(range(NCORES)), trace=TRACE)
    _CACHE["last_result"] = res

    ctx = np.concatenate([r["ctx_out"] for r in res.results], axis=0)
    attnT = np.concatenate([r["attnT_out"] for r in res.results], axis=0)
    ncumT = np.concatenate([r["cumT_out"] for r in res.results], axis=0)
    attn = np.ascontiguousarray(attnT.transpose(0, 2, 1)).reshape(B, T)
    new_cum = np.ascontiguousarray(ncumT.transpose(0, 2, 1)).reshape(B, T)
    return ctx, attn, new_cum


# revision 22
# speedup vs baseline: 61223.0240x; 2.6600x over previous
"""Location-sensitive attention on 8 Trainium2 NeuronCores.

Strategy (data-parallel over batch, 4 rows/core):
  - Host stages the big memory tensor per-core in two fp16 layouts:
      memT16 [4, E, T]  (E-major, feeds the W_memory projection: contraction
                         over E must sit on SBUF partitions)
      mem16  [4, T, E]  (natural, feeds the context matvec: contraction over T)
    Total HBM traffic per core = 2 x 16 MiB = 32 MiB, the same byte count as
    reading the fp32 input once; all matmul contractions land on partitions
    with zero on-chip transposes of the big tensor.
  - Energies are accumulated in PSUM [A=128, T-tile] (pm + conv-location
    projection), tanh+query-bias applied by ScalarE, reduced against W_value
    by TensorE into [T,1] columns, softmax done in a [128, 32] layout with a
    cross-partition renorm via two tiny PE transposes.
  - conv(prev_attn, prev_attn_cum) is folded with W_loc on the host into a
    single [62, A] projection applied to a 62-row stack of shifted windows,
    built by overlapping-read DMAs from host-padded rows.
"""

import numpy as np

import concourse.bass as bass
import concourse.tile as tile
from concourse import bacc, mybir
from concourse.bass_utils import run_bass_kernel_spmd

B, T, E, Q = 32, 4096, 512, 1024
A, F, KW = 128, 32, 31
NCORES = 8
BPC = B // NCORES          # batch rows per core
PAD = 15                   # SAME conv padding for kernel width 31
TPADLEN = T + 2 * PAD + 2  # 4128, padded to a multiple of 32
NG = T // 128              # 32 column-groups in the [128, NG] energy layout
NSUPER = T // 512          # 8 T-supertiles of 512
f32 = mybir.dt.float32
f16 = mybir.dt.float16

TRACE = False              # test.py flips this to collect an NTFF profile
_CACHE = {}


def _build(reps=1, ablate=(), mode="2pass"):
    cache_mode = mode == "cache"
    no_pass2 = "pass2" in ablate
    no_pass1mm = "pass1mm" in ablate
    no_cat = "cat" in ablate
    nc = bacc.Bacc(
        "TRN2", target_bir_lowering=False, debug=False, num_devices=NCORES
    )

    memT16 = nc.dram_tensor("memT16", [BPC, E, T], f16, kind="ExternalInput")
    mem16 = nc.dram_tensor("mem16", [BPC, T, E], f16, kind="ExternalInput")
    wm16 = nc.dram_tensor("wm16", [128, 4, 128], f16, kind="ExternalInput")
    wcomb = nc.dram_tensor("wcomb", [2, 31, A], f16, kind="ExternalInput")
    pqT = nc.dram_tensor("pqT", [A, BPC], f32, kind="ExternalInput")
    wv = nc.dram_tensor("wv", [A, 1], f16, kind="ExternalInput")
    prevpad = nc.dram_tensor("prevpad", [BPC, TPADLEN], f16, kind="ExternalInput")
    cumpad = nc.dram_tensor("cumpad", [BPC, TPADLEN], f16, kind="ExternalInput")
    maskb = nc.dram_tensor("maskb", [BPC, 128, NG], f32, kind="ExternalInput")
    cumT = nc.dram_tensor("cumT", [BPC, 128, NG], f32, kind="ExternalInput")
    ident = nc.dram_tensor("ident", [128, 128], f32, kind="ExternalInput")

    ctx_out = nc.dram_tensor("ctx_out", [BPC, E], f32, kind="ExternalOutput")
    attnT_out = nc.dram_tensor("attnT_out", [BPC, 128, NG], f32, kind="ExternalOutput")
    cumT_out = nc.dram_tensor("cumT_out", [BPC, 128, NG], f32, kind="ExternalOutput")

    with tile.TileContext(nc) as tc:
        with (
            tc.tile_pool(name="consts", bufs=1) as consts,
            tc.tile_pool(name="memt", bufs=(16 if cache_mode else 10)) as memt_pool,
            tc.tile_pool(name="mem2", bufs=6) as mem2_pool,
            tc.tile_pool(name="scr", bufs=2) as scr_pool,
            tc.tile_pool(name="catp", bufs=2) as cat_pool,
            tc.tile_pool(name="zp", bufs=3) as z_pool,
            tc.tile_pool(name="small", bufs=2) as small,
            tc.tile_pool(name="ps_pm", bufs=2, space="PSUM") as ps_pm,
            tc.tile_pool(name="ps_e", bufs=2, space="PSUM") as ps_e,
            tc.tile_pool(name="ps_ctx", bufs=2, space="PSUM") as ps_ctx,
            tc.tile_pool(name="ps_st", bufs=2, space="PSUM") as ps_st,
        ):
            wm_sb = consts.tile([128, 4, 128], f16)
            nc.gpsimd.dma_start(out=wm_sb, in_=wm16[:, :, :])
            wc_sb = consts.tile([31, 2, A], f16)
            nc.gpsimd.dma_start(out=wc_sb, in_=wcomb.rearrange("c k a -> k c a"))
            pq_sb = consts.tile([A, BPC], f32)
            nc.gpsimd.dma_start(out=pq_sb, in_=pqT[:, :])
            wv_sb = consts.tile([A, 1], f16)
            nc.gpsimd.dma_start(out=wv_sb, in_=wv[:, :])
            id_sb = consts.tile([128, 128], f32)
            nc.gpsimd.dma_start(out=id_sb, in_=ident[:, :])
            mk_sb = consts.tile([128, BPC, NG], f32)
            nc.gpsimd.dma_start(out=mk_sb, in_=maskb.rearrange("b p g -> p b g"))
            cm_sb = consts.tile([128, BPC, NG], f32)
            nc.gpsimd.dma_start(out=cm_sb, in_=cumT.rearrange("b p g -> p b g"))
            ones_sb = consts.tile([1, 128], f32)
            nc.vector.memset(ones_sb, 1.0)

            for b in [b for _ in range(reps) for b in range(BPC)]:
                # ---------------- pass 1: energies ----------------
                energ = small.tile([128, NG], f32, tag="energ")
                mts_all = []
                for half in range(2):
                    t0 = 2048 * half
                    mts = []
                    for j in range(4):
                        mt = memt_pool.tile([128, 2048], f16, tag="mt")
                        eng = nc.sync if (not cache_mode or j < 2) else nc.scalar
                        eng.dma_start(
                            out=mt, in_=memT16[b, 128 * j:128 * (j + 1), t0:t0 + 2048]
                        )
                        mts.append(mt)
                    mts_all.append(mts)
                    cat = cat_pool.tile([31, 2, 2048], f16, tag="cat")
                    if not no_cat:
                        for ci, src_t in enumerate((prevpad, cumpad)):
                            base = src_t[b, t0:t0 + 2048]
                            ov = bass.AP(
                                tensor=base.tensor,
                                offset=base.offset,
                                ap=[[1, 31], [1, 2048]],
                            )
                            nc.gpsimd.dma_start(out=cat[:, ci, :], in_=ov)
                    for g4 in range(4):
                        sup = 4 * half + g4
                        c0 = 512 * g4
                        pm = ps_pm.tile([128, 512], f32, tag="pm")
                        if no_pass1mm:
                            nc.tensor.matmul(
                                pm, wm_sb[:, 0, :], mts[0][:, c0:c0 + 512],
                                start=True, stop=True,
                            )
                        else:
                            for j in range(4):
                                nc.tensor.matmul(
                                    pm, wm_sb[:, j, :], mts[j][:, c0:c0 + 512],
                                    start=(j == 0), stop=False,
                                )
                            for ci in range(2):
                                nc.tensor.matmul(
                                    pm, wc_sb[:, ci, :], cat[:, ci, c0:c0 + 512],
                                    start=False, stop=(ci == 1),
                                )
                        z = z_pool.tile([128, 512], f16, tag="z")
                        nc.scalar.activation(
                            out=z, in_=pm, func=mybir.ActivationFunctionType.Tanh,
                            bias=pq_sb[:, b:b + 1],
                        )
                        pe4 = ps_e.tile([128, 4], f32, tag="pe")
                        for s in range(4):
                            nc.tensor.matmul(
                                pe4[:, s:s + 1], z[:, 128 * s:128 * (s + 1)], wv_sb,
                                start=(s == 0), stop=(s == 3),
                            )
                        nc.vector.tensor_copy(
                            out=energ[:, 4 * sup:4 * sup + 4], in_=pe4
                        )

                # ---------------- softmax over T (layout [128, NG]) ----------
                nc.vector.tensor_add(energ, energ, mk_sb[:, b, :])
                mrow = small.tile([128, 1], f32, tag="mrow")
                nc.vector.reduce_max(out=mrow, in_=energ, axis=mybir.AxisListType.X)
                negm = small.tile([128, 1], f32, tag="negm")
                nc.vector.tensor_scalar_mul(negm, mrow, -1.0)
                ssum = small.tile([128, 1], f32, tag="ssum")
                eexp = small.tile([128, NG], f32, tag="eexp")
                nc.scalar.activation(
                    out=eexp, in_=energ, func=mybir.ActivationFunctionType.Exp,
                    bias=negm, accum_out=ssum,
                )
                pn = ps_st.tile([1, 128], f32, tag="st")
                nc.tensor.transpose(pn, negm, id_sb)
                psrow = ps_st.tile([1, 128], f32, tag="st")
                nc.tensor.transpose(psrow, ssum, id_sb)
                nrow = small.tile([1, 128], f32, tag="nrow")
                nc.vector.tensor_copy(out=nrow, in_=pn)
                srow = small.tile([1, 128], f32, tag="srow")
                nc.vector.tensor_copy(out=srow, in_=psrow)
                minneg = small.tile([1, 1], f32, tag="minneg")
                nc.vector.tensor_reduce(
                    out=minneg, in_=nrow, axis=mybir.AxisListType.X,
                    op=mybir.AluOpType.min,
                )
                wrow = small.tile([1, 128], f32, tag="wrow")
                nc.scalar.activation(
                    out=wrow, in_=nrow,
                    func=mybir.ActivationFunctionType.Exp,
                    bias=minneg, scale=-1.0,
                )
                swrow = small.tile([1, 128], f32, tag="swrow")
                nc.vector.tensor_mul(swrow, wrow, srow)
                stot = small.tile([1, 1], f32, tag="stot")
                nc.vector.reduce_sum(
                    out=stot, in_=swrow, axis=mybir.AxisListType.X
                )
                rinv = small.tile([1, 1], f32, tag="rinv")
                nc.vector.reciprocal(out=rinv, in_=stot)
                nc.vector.tensor_scalar_mul(wrow, wrow, rinv)
                pf = ps_st.tile([128, 1], f32, tag="st")
                nc.tensor.transpose(pf, wrow, id_sb[0:1, 0:1])
                fcol = small.tile([128, 1], f32, tag="fcol")
                nc.vector.tensor_copy(out=fcol, in_=pf)

                attnf = small.tile([128, NG], f32, tag="attnf")
                nc.vector.tensor_scalar_mul(attnf, eexp, fcol)
                nc.gpsimd.dma_start(out=attnT_out[b, :, :], in_=attnf)
                attn16 = small.tile([128, NG], f16, tag="attn16")
                nc.vector.tensor_copy(out=attn16, in_=attnf)
                ncum = small.tile([128, NG], f32, tag="ncum")
                nc.vector.tensor_add(ncum, attnf, cm_sb[:, b, :])
                nc.gpsimd.dma_start(out=cumT_out[b, :, :], in_=ncum)

                # ---------------- pass 2: context ----------------
                if no_pass2:
                    continue
                if cache_mode:
                    # attn row layout [1, T] via transpose + SBUF linearize DMA
                    pat = ps_st.tile([NG, 128], f32, tag="st")
                    nc.tensor.transpose(pat, attnf, id_sb)
                    arows = small.tile([NG, 128], f32, tag="arows")
                    nc.vector.tensor_copy(out=arows, in_=pat)
                    arow = small.tile([1, T], f32, tag="arow")
                    nc.gpsimd.dma_start(
                        out=arow, in_=arows.rearrange("g p -> (g p)")[None, :]
                    )
                    accs = []
                    for j in range(4):
                        acc = small.tile([128, 1], f32, tag=f"acc{j}")
                        accs.append(acc)
                    for seg in range(8):
                        rep = ps_ctx.tile([128, 512], f32, tag="rep")
                        nc.tensor.matmul(
                            rep, ones_sb, arow[0:1, 512 * seg:512 * (seg + 1)],
                            start=True, stop=True,
                        )
                        for j in range(4):
                            scratch = scr_pool.tile([128, 512], f32, tag="scr")
                            nc.vector.tensor_tensor_reduce(
                                out=scratch,
                                in0=mts_all[seg // 4][j][:, 512 * (seg % 4):512 * (seg % 4 + 1)],
                                in1=rep,
                                scale=1.0,
                                scalar=(0.0 if seg == 0 else accs[j]),
                                op0=mybir.AluOpType.mult,
                                op1=mybir.AluOpType.add,
                                accum_out=accs[j],
                            )
                    ctxc = small.tile([128, 4], f32, tag="ctxc")
                    for j in range(4):
                        nc.vector.tensor_copy(out=ctxc[:, j:j + 1], in_=accs[j])
                    nc.gpsimd.dma_start(
                        out=ctx_out[b, :].rearrange("(j p) -> p j", p=128), in_=ctxc
                    )
                    continue
                pc = ps_ctx.tile([1, 512], f32, tag="pc")
                nat = mem16[b, :, :].rearrange("(g t) e -> t g e", t=128)
                for h2 in range(4):
                    m2 = mem2_pool.tile([128, 8, 512], f16, tag="m2")
                    nc.scalar.dma_start(out=m2, in_=nat[:, 8 * h2:8 * h2 + 8, :])
                    for i in range(8):
                        g = 8 * h2 + i
                        nc.tensor.matmul(
                            pc, attn16[:, g:g + 1], m2[:, i, :],
                            start=(g == 0), stop=(g == 31),
                        )
                ctx_sb = small.tile([1, 512], f32, tag="ctx_sb")
                nc.vector.tensor_copy(out=ctx_sb, in_=pc)
                nc.gpsimd.dma_start(out=ctx_out[b, :], in_=ctx_sb)

    nc.compile()
    return nc


def _get_nc(reps=1):
    key = ("nc", reps)
    if key not in _CACHE:
        _CACHE[key] = _build(reps)
    return _CACHE[key]


def _stage_inputs(query, memory, mask, prev_attn, prev_attn_cum,
                  W_query, W_memory, W_value, conv_kernel, W_loc):
    query = np.asarray(query, np.float32)
    memory = np.asarray(memory, np.float32)
    mask = np.asarray(mask)
    prev_attn = np.asarray(prev_attn, np.float32)
    prev_attn_cum = np.asarray(prev_attn_cum, np.float32)
    W_query = np.asarray(W_query, np.float32)
    W_memory = np.asarray(W_memory, np.float32)
    W_value = np.asarray(W_value, np.float32)
    conv_kernel = np.asarray(conv_kernel, np.float32)
    W_loc = np.asarray(W_loc, np.float32)

    # Tiny host-side staging: query projection, conv+W_loc fold, padding and
    # the [128, NG] relayouts that match the on-chip energy layout.
    pq = query @ W_query                                   # [B, A]
    wcomb = np.einsum("kcf,fa->cka", conv_kernel, W_loc).astype(np.float16)
    wm16 = np.ascontiguousarray(
        W_memory.astype(np.float16).reshape(4, 128, 128).transpose(1, 0, 2)
    )
    mem16 = memory.astype(np.float16)                      # [B, T, E]
    memT16 = memory.transpose(0, 2, 1).astype(np.float16)  # [B, E, T]
    prevpad = np.zeros((B, TPADLEN), np.float16)
    prevpad[:, PAD:PAD + T] = prev_attn
    cumpad = np.zeros((B, TPADLEN), np.float16)
    cumpad[:, PAD:PAD + T] = prev_attn_cum
    fmin = np.float32(np.finfo(np.float32).min)
    maskb = np.ascontiguousarray(
        np.where(mask, np.float32(0.0), fmin).reshape(B, NG, 128).transpose(0, 2, 1)
    )
    cumT = np.ascontiguousarray(
        prev_attn_cum.reshape(B, NG, 128).transpose(0, 2, 1)
    )
    ident = np.eye(128, dtype=np.float32)
    wv = np.ascontiguousarray(W_value.reshape(A, 1).astype(np.float16))

    in_maps = []
    for c in range(NCORES):
        sl = slice(c * BPC, (c + 1) * BPC)
        in_maps.append({
            "memT16": np.ascontiguousarray(memT16[sl]),
            "mem16": np.ascontiguousarray(mem16[sl]),
            "wm16": wm16,
            "wcomb": np.ascontiguousarray(wcomb),
            "pqT": np.ascontiguousarray(pq[sl].T),
            "wv": wv,
            "prevpad": np.ascontiguousarray(prevpad[sl]),
            "cumpad": np.ascontiguousarray(cumpad[sl]),
            "maskb": maskb[sl],
            "cumT": cumT[sl],
            "ident": ident,
        })
    return in_maps


def kernel(query, memory, mask, prev_attn, prev_attn_cum,
           W_query, W_memory, W_value, conv_kernel, W_loc):
    in_maps = _stage_inputs(query, memory, mask, prev_attn, prev_attn_cum,
                            W_query, W_memory, W_value, conv_kernel, W_loc)
    nc = _get_nc()
    res = run_bass_kernel_spmd(nc, in_maps, list(range(NCORES)), trace=TRACE)
    _CACHE["last_result"] = res

    ctx = np.concatenate([r["ctx_out"] for r in res.results], axis=0)
    attnT = np.concatenate([r["attnT_out"] for r in res.results], axis=0)
    ncumT = np.concatenate([r["cumT_out"] for r in res.results], axis=0)
    attn = np.ascontiguousarray(attnT.transpose(0, 2, 1)).reshape(B, T)
    new_cum = np.ascontiguousarray(ncumT.transpose(0, 2, 1)).reshape(B, T)
    return ctx, attn, new_cum
